# revision 25
# baseline (speedup 1.0000x reference)
"""Trainium2 Bass kernel for LocalCrossCorrelationWithSmoothnessLoss (v2).

Full inputs in, full output out. Pure data-parallel over batch (B=8 -> 8
NeuronCores); each core computes partial sums for its image; host combines.

Per-core pipeline (one 1024x1024 image pair + two flow channels):
  load       I, J, s loaded as bf16 via gpsimd cast-DMA (f32 in HBM).
  products   IJ (DVE), I2 (DVE), J2 (ACT), all bf16, 9 h-chunks w/ halo.
  stage A    fused H-conv + transpose: one matmul per (w-chunk, map,
             h-chunk) with the product map as the STATIONARY operand and
             the 9-tap band as the moving operand -> psum [w, h] f32.
             Product maps use an 81-scaled band.
  T copy     psum f32 -> SBUF bf16 T maps (DVE/ACT split).
  stage B    W-conv: band stationary [in2,on2], T moving -> psum
             [w_out, h] f32 per map.
  combine    crossN = S_IJ81 - S_I*S_J, IvarN = S_II81 - S_I^2,
             JvarN = S_JJ81 - S_J^2, cc = (crossN * rsqrt(IvarN*JvarN))^2
             accumulated per-partition via ACT Square accum_out.
  smooth     s^2 (ACT accum), lag_w via DVE STT accum; lag_h via
             transpose-mode PE row-shift into bf16 psum + DVE STT accum.
             Interleaved into the c2 sweep for overlap.

Output per core: partial sums vector. Host assembles losses in float64.
"""
import sys
import numpy as np

sys.path.insert(0, "/opt/trn_rl_repo")

import ml_dtypes
import bass_rust
import concourse.bass as bass
import concourse.tile as tile
from concourse import mybir
from concourse import bass_utils
from concourse import tile_utils

F32 = mybir.dt.float32
BF16 = mybir.dt.bfloat16
ALU = mybir.AluOpType
ACTF = mybir.ActivationFunctionType

H = 1024
W = 1024
PAD = 4
ALPHA = 0.01
STRIDE = 120

# chunk table: (out_lo, out_n, in_lo, in_n) -- shared by h- and w-chunking
CHUNKS = []
for _c in range((H + STRIDE - 1) // STRIDE):
    _olo = STRIDE * _c
    _on = min(STRIDE, H - _olo)
    _ilo = max(0, _olo - PAD)
    _ihi = min(H, _olo + _on + PAD)
    CHUNKS.append((_olo, _on, _ilo, _ihi - _ilo))
NCH = len(CHUNKS)

# accumulator columns
CC0 = 0            # 9 cols: cc sum per w-chunk
SQ0 = 9            # 16 cols: sum s^2 per tile
LW0 = 25           # 16 cols: lag_w per tile
LH0 = 41           # 32 cols: lag_h per (tile, half)
NCOL = 73

tile_utils.max_sbuf_usage = 206 * 1024

_nc_cache = {}


def _legalize_waits(nc, max_waits=1):
    """walrus accepts only one sync-wait per instruction; split extras onto
    same-engine NoOps placed just before."""
    ctr = 0
    for f in nc.m.functions:
        for bb in f.blocks:
            insts = bb.instructions
            i = 0
            while i < len(insts):
                ins = insts[i]
                si = ins.sync_info
                if si is None:
                    i += 1
                    continue
                w = list(si.on_wait)
                if len(w) <= max_waits:
                    i += 1
                    continue
                extra, keep = w[:-max_waits], w[-max_waits:]
                nops = []
                for j in range(0, len(extra), max_waits):
                    chunk = extra[j:j + max_waits]
                    nop = mybir.InstNoOp(name=f"I-wsplit-{ctr}", ins=[], outs=[])
                    ctr += 1
                    nop.engine = ins.engine
                    nop.sync_info = bass_rust.SyncInfo(on_wait=chunk, on_update=[])
                    nops.append(nop)
                ins.sync_info = bass_rust.SyncInfo(on_wait=keep,
                                                  on_update=list(si.on_update))
                insts[i:i] = nops
                i += len(nops) + 1


def _make_host_consts():
    """Band matrices (bf16) and ones (f32). Layout [128, 4*STRIDE]:
    v0 = chunk-0 band, v1 = interior band, v0s/v1s = 81-scaled."""
    def band(klo, kn, olo, on, scale):
        k = np.arange(klo, klo + kn)[:, None]
        m = np.arange(olo, olo + on)[None, :]
        return (np.abs(k - m) <= PAD).astype(np.float32) * scale

    bands = np.zeros((128, 4 * STRIDE), dtype=np.float32)
    olo0, on0, ilo0, in0 = CHUNKS[0]
    olo1, on1, ilo1, in1 = CHUNKS[1]
    bands[:in0, 0:on0] = band(ilo0, in0, olo0, on0, 1.0)
    bands[:in1, STRIDE:STRIDE + on1] = band(ilo1, in1, olo1, on1, 1.0)
    bands[:in0, 2 * STRIDE:2 * STRIDE + on0] = band(ilo0, in0, olo0, on0, 81.0)
    bands[:in1, 3 * STRIDE:3 * STRIDE + on1] = band(ilo1, in1, olo1, on1, 81.0)
    bands_bf = bands.astype(ml_dtypes.bfloat16)
    shift = np.zeros((128, 127), dtype=np.float32)
    shift[np.arange(1, 128), np.arange(127)] = 1.0
    shift_bf = shift.astype(ml_dtypes.bfloat16)
    ones_f32 = np.ones((128, 1), dtype=np.float32)
    return bands_bf, shift_bf, ones_f32


def _band_ap(bands_t, c, scaled, inn, on):
    v = (0 if c == 0 else 1) + (2 if scaled else 0)
    return bands_t[0:inn, v * STRIDE:v * STRIDE + on]


def _build(nc):
    I_d = nc.dram_tensor("I", [H, W], F32, kind="ExternalInput").ap()
    J_d = nc.dram_tensor("J", [H, W], F32, kind="ExternalInput").ap()
    s0_d = nc.dram_tensor("s0", [H, W], F32, kind="ExternalInput").ap()
    s1_d = nc.dram_tensor("s1", [H, W], F32, kind="ExternalInput").ap()
    bands_d = nc.dram_tensor("bands", [128, 4 * STRIDE], BF16,
                             kind="ExternalInput").ap()
    shift_d = nc.dram_tensor("shift", [128, 127], BF16,
                             kind="ExternalInput").ap()
    ones_d = nc.dram_tensor("ones", [128, 1], F32, kind="ExternalInput").ap()
    part_d = nc.dram_tensor("partials", [1, NCOL], F32,
                            kind="ExternalOutput").ap()

    from contextlib import ExitStack
    with tile.TileContext(nc) as tc, ExitStack() as ctx:
        consts = ctx.enter_context(tc.tile_pool(name="consts", bufs=1))
        prod = ctx.enter_context(tc.tile_pool(name="prod", bufs=1))
        tmap = ctx.enter_context(tc.tile_pool(name="tmap", bufs=2))
        ctmp = ctx.enter_context(tc.tile_pool(name="ctmp", bufs=2))
        spool = ctx.enter_context(tc.tile_pool(name="spool", bufs=1))
        accp = ctx.enter_context(tc.tile_pool(name="accp", bufs=1))
        psA = ctx.enter_context(tc.tile_pool(name="psA", bufs=2, space="PSUM"))
        ps2 = ctx.enter_context(tc.tile_pool(name="ps2", bufs=2, space="PSUM"))

        bands_t = consts.tile([128, 4 * STRIDE], BF16)
        shift_t = consts.tile([128, 127], BF16)
        ones_t = consts.tile([128, 1], F32)
        nc.sync.dma_start(bands_t[:], bands_d)
        nc.sync.dma_start(shift_t[:], shift_d)
        nc.sync.dma_start(ones_t[:], ones_d)

        acc = accp.tile([128, NCOL], F32)
        nc.vector.memset(acc[:], 0.0)

        # ---------------- phase P: load + products (bf16) ----------------
        # maps: 0=I, 1=J (unscaled band), 2=IJ, 3=II, 4=JJ (81-scaled band)
        pmaps = [[None] * NCH for _ in range(5)]
        for ch, (olo, on, ilo, inn) in enumerate(CHUNKS):
            Ib = prod.tile([128, W], BF16, tag=f"Ib{ch}", name=f"Ib{ch}")
            Jb = prod.tile([128, W], BF16, tag=f"Jb{ch}", name=f"Jb{ch}")
            nc.gpsimd.dma_start(Ib[0:inn, :], I_d[ilo:ilo + inn, :])
            nc.gpsimd.dma_start(Jb[0:inn, :], J_d[ilo:ilo + inn, :])
            IJ = prod.tile([128, W], BF16, tag=f"IJ{ch}", name=f"IJ{ch}")
            I2 = prod.tile([128, W], BF16, tag=f"I2{ch}", name=f"I2{ch}")
            J2 = prod.tile([128, W], BF16, tag=f"J2{ch}", name=f"J2{ch}")
            nc.vector.tensor_tensor(out=IJ[0:inn, :], in0=Ib[0:inn, :],
                                    in1=Jb[0:inn, :], op=ALU.mult)
            nc.scalar.square(I2[0:inn, :], Ib[0:inn, :])
            nc.scalar.square(J2[0:inn, :], Jb[0:inn, :])
            pmaps[0][ch] = Ib
            pmaps[1][ch] = Jb
            pmaps[2][ch] = IJ
            pmaps[3][ch] = I2
            pmaps[4][ch] = J2

        # s loads early so DMA overlaps compute; tiles persist.
        s_tiles = []
        for ch_i, s_d in enumerate((s0_d, s1_d)):
            for t in range(8):
                st = spool.tile([128, W], BF16, tag=f"s{ch_i}_{t}",
                                name=f"s{ch_i}_{t}")
                nc.gpsimd.dma_start(st[:], s_d[128 * t:128 * (t + 1), :])
                s_tiles.append(st)

        # smoothness for one s tile; lag_h via transpose-mode PE row-shift
        # (bf16 psum), STT accumulate. Interleaved into the c2 sweep.
        def emit_smooth(idx):
            st = s_tiles[idx]
            s2o = ctmp.tile([128, W], BF16, tag="junk", bufs=3, name="s2o")
            nc.scalar.activation(s2o[:], st[:], ACTF.Square,
                                 accum_out=acc[:, SQ0 + idx:SQ0 + idx + 1])
            lw = ctmp.tile([128, W], BF16, tag="junk", bufs=3, name="lw")
            nc.vector.scalar_tensor_tensor(
                out=lw[:, 0:W - 1], in0=st[:, 1:W], scalar=1.0,
                in1=st[:, 0:W - 1], op0=ALU.mult, op1=ALU.mult,
                accum_out=acc[:, LW0 + idx:LW0 + idx + 1])
            psh = ps2.tile([128, W], BF16, tag="p2f", name="psh")
            for blk in range(8):
                sl = slice(128 * blk, 128 * blk + 128)
                nc.tensor.matmul(psh[0:127, sl], shift_t[:], st[:, sl],
                                 is_transpose=True,
                                 start=(blk == 0), stop=(blk == 7),
                                 skip_group_check=True)
            lh = ctmp.tile([128, W], BF16, tag="junk", bufs=3, name="lh")
            nc.vector.scalar_tensor_tensor(
                out=lh[0:127, :], in0=psh[0:127, :], scalar=1.0,
                in1=st[0:127, :], op0=ALU.mult, op1=ALU.mult,
                accum_out=acc[0:127, LH0 + idx:LH0 + idx + 1])

        # ------------- phase AB per w-chunk c2 -------------------------
        for c2, (olo2, on2, ilo2, in2) in enumerate(CHUNKS):
            # stage A: fused H-conv + transpose -> T maps [w, h] bf16
            # one 2-bank f32 psum tile; per-bank start/stop groups
            t_tiles = []
            for m in range(5):
                pA = psA.tile([128, W], F32, tag="psA", name="pA")
                scaled = m >= 2
                first = {0: True, 1: True}
                # writes: (bank, col_lo, col_hi, ch, band_lo, band_hi)
                writes = []
                for ch, (holo, hon, hilo, hinn) in enumerate(CHUNKS):
                    lo, hi = holo, holo + hon
                    if hi <= 512 or lo >= 512:
                        writes.append((0 if hi <= 512 else 1, lo, hi, ch, 0, hon))
                    else:
                        writes.append((0, lo, 512, ch, 0, 512 - lo))
                        writes.append((1, 512, hi, ch, 512 - lo, hon))
                lastbank = {}
                for i, wr in enumerate(writes):
                    lastbank[wr[0]] = i
                for i, (bk, lo, hi, ch, blo, bhi) in enumerate(writes):
                    _, hon, hilo, hinn = CHUNKS[ch]
                    nc.tensor.matmul(
                        pA[0:in2, lo:hi],
                        pmaps[m][ch][0:hinn, ilo2:ilo2 + in2],
                        _band_ap(bands_t, ch, scaled, hinn, CHUNKS[ch][1])[:, blo:bhi],
                        start=first[bk], stop=(lastbank[bk] == i),
                        skip_group_check=True)
                    first[bk] = False
                tt = tmap.tile([128, W], BF16, tag=f"T{m}", name=f"T{m}")
                if (c2 * 5 + m) % 3 == 0:
                    nc.vector.tensor_copy(tt[0:in2, :], pA[0:in2, :])
                else:
                    nc.scalar.copy(tt[0:in2, :], pA[0:in2, :])
                t_tiles.append(tt)

            # stage B: W-conv, band stationary, T moving -> psum f32
            # combine consumes each map psum; si copied (used twice)
            bw = _band_ap(bands_t, c2, False, in2, on2)
            p2 = []
            for m in range(5):
                ph = ps2.tile([128, W], F32, tag="p2f", name="ph")
                nc.tensor.matmul(ph[0:on2, 0:512], bw, t_tiles[m][0:in2, 0:512],
                                 start=True, stop=True, skip_group_check=True)
                nc.tensor.matmul(ph[0:on2, 512:1024], bw,
                                 t_tiles[m][0:in2, 512:1024],
                                 start=True, stop=True, skip_group_check=True)
                p2.append(ph)

            n = on2
            # si copied to sbuf (read 2x); A = si^2 on ACT
            si = ctmp.tile([128, W], BF16, tag="si")
            nc.vector.tensor_copy(si[0:n, :], p2[0][0:n, :])
            A = ctmp.tile([128, W], BF16, tag="A")
            nc.scalar.square(A[0:n, :], si[0:n, :])
            # B = sj^2 straight from psum on ACT
            B = ctmp.tile([128, W], BF16, tag="B")
            nc.scalar.square(B[0:n, :], p2[1][0:n, :])
            # P = si * sj (one psum operand)
            P = ctmp.tile([128, W], BF16, tag="P")
            nc.vector.tensor_tensor(out=P[0:n, :], in0=si[0:n, :],
                                    in1=p2[1][0:n, :], op=ALU.mult)
            # crossN / IvarN / JvarN: psum - sbuf subtracts on DVE
            crossN = ctmp.tile([128, W], BF16, tag="crossN", bufs=3)
            nc.vector.tensor_tensor(out=crossN[0:n, :], in0=p2[2][0:n, :],
                                    in1=P[0:n, :], op=ALU.subtract)
            IvarN = ctmp.tile([128, W], BF16, tag="IvarN", bufs=3)
            nc.vector.tensor_tensor(out=IvarN[0:n, :], in0=p2[3][0:n, :],
                                    in1=A[0:n, :], op=ALU.subtract)
            JvarN = ctmp.tile([128, W], BF16, tag="JvarN", bufs=3)
            nc.vector.tensor_tensor(out=JvarN[0:n, :], in0=p2[4][0:n, :],
                                    in1=B[0:n, :], op=ALU.subtract)
            # D = IvarN * JvarN (bf16); ln kept in f32 for exp accuracy
            D = ctmp.tile([128, W], BF16, tag="D")
            nc.vector.tensor_tensor(out=D[0:n, :], in0=IvarN[0:n, :],
                                    in1=JvarN[0:n, :], op=ALU.mult)
            lnD = ctmp.tile([128, W], F32, tag="lnD")
            nc.scalar.activation(lnD[0:n, :], D[0:n, :], ACTF.Ln)
            t_r = ctmp.tile([128, W], BF16, tag="t_r", bufs=3)
            nc.scalar.activation(t_r[0:n, :], lnD[0:n, :], ACTF.Exp, scale=-0.5)
            u = ctmp.tile([128, W], BF16, tag="u", bufs=3)
            nc.vector.tensor_tensor(out=u[0:n, :], in0=crossN[0:n, :],
                                    in1=t_r[0:n, :], op=ALU.mult)
            ujunk = ctmp.tile([128, W], BF16, tag="junk", bufs=3)
            nc.scalar.activation(ujunk[0:n, :], u[0:n, :], ACTF.Square,
                                 accum_out=acc[0:n, CC0 + c2:CC0 + c2 + 1])

            if c2 < 8:
                emit_smooth(2 * c2)
                emit_smooth(2 * c2 + 1)

        # ---------------- final partition reduction ---------------------
        pF = ps2.tile([1, NCOL], F32, tag="p2f", name="pF")
        nc.tensor.matmul(pF[:], ones_t[:], acc[:], start=True, stop=True)
        outt = accp.tile([1, NCOL], F32, tag="outt")
        nc.scalar.copy(outt[:], pF[:])
        nc.sync.dma_start(part_d, outt[:])

    return


def _get_nc():
    if "nc" not in _nc_cache:
        nc = bass.Bass("TRN2", target_bir_lowering=False, debug=False)
        _build(nc)
        _legalize_waits(nc)
        _nc_cache["nc"] = nc
    return _nc_cache["nc"]


def _in_maps(I, J, s):
    B = I.shape[0]
    bands_bf, shift_bf, ones_f32 = _make_host_consts()
    in_maps = []
    for b in range(B):
        in_maps.append({
            "I": np.ascontiguousarray(I[b, 0]),
            "J": np.ascontiguousarray(J[b, 0]),
            "s0": np.ascontiguousarray(s[b, 0]),
            "s1": np.ascontiguousarray(s[b, 1]),
            "bands": bands_bf,
            "shift": shift_bf,
            "ones": ones_f32,
        })
    return in_maps


def kernel(I, J, s, sum_filt):
    B = I.shape[0]
    assert I.shape == (B, 1, H, W) and s.shape == (B, 2, H, W)
    nc = _get_nc()
    res = bass_utils.run_bass_kernel_spmd(nc, _in_maps(I, J, s),
                                          core_ids=list(range(B)))
    parts = np.stack([res.results[b]["partials"][0] for b in range(B)])
    parts = parts.astype(np.float64)

    s64 = s.astype(np.float64)
    cc_sum = float(parts[:, CC0:CC0 + 9].sum())
    s2 = parts[:, SQ0:SQ0 + 16].sum(axis=1)
    lag_w = parts[:, LW0:LW0 + 16].sum(axis=1)
    lag_h = parts[:, LH0:LH0 + 16].sum(axis=1)

    # tile-boundary lag_h pairs (rows 127/128, ...) per core
    rb = np.arange(127, H - 1, 128)
    lag_h = lag_h + (s64[:, :, rb, :] * s64[:, :, rb + 1, :]).sum(axis=(1, 2, 3))

    # edge corrections per core (both channels folded together)
    e_w = (s64[:, :, :, 0] ** 2).sum(axis=(1, 2)) + \
          (s64[:, :, :, -1] ** 2).sum(axis=(1, 2))
    e_h = (s64[:, :, 0, :] ** 2).sum(axis=(1, 2)) + \
          (s64[:, :, -1, :] ** 2).sum(axis=(1, 2))

    sum_dx2 = (2.0 * s2 - e_w - 2.0 * lag_w).sum()
    sum_dy2 = (2.0 * s2 - e_h - 2.0 * lag_h).sum()
    cnt = B * 2 * H * (W - 1)

    ncc_loss = -cc_sum / (B * H * W)
    smooth = 0.5 * (sum_dx2 / cnt + sum_dy2 / cnt) * ALPHA
    total = ncc_loss + smooth
    return np.array([total, ncc_loss, smooth], dtype=np.float32)


# revision 26
# speedup vs baseline: 1.0157x; 1.0157x over previous
"""Trainium2 Bass kernel for LocalCrossCorrelationWithSmoothnessLoss (v2).

Full inputs in, full output out. Pure data-parallel over batch (B=8 -> 8
NeuronCores); each core computes partial sums for its image; host combines.

Per-core pipeline (one 1024x1024 image pair + two flow channels):
  load       I, J, s loaded as bf16 via gpsimd cast-DMA (f32 in HBM).
  products   IJ (DVE), I2 (DVE), J2 (ACT), all bf16, 9 h-chunks w/ halo.
  stage A    fused H-conv + transpose: one matmul per (w-chunk, map,
             h-chunk) with the product map as the STATIONARY operand and
             the 9-tap band as the moving operand -> psum [w, h] f32.
             Product maps use an 81-scaled band.
  T copy     psum f32 -> SBUF bf16 T maps (DVE/ACT split).
  stage B    W-conv: band stationary [in2,on2], T moving -> psum
             [w_out, h] f32 per map.
  combine    crossN = S_IJ81 - S_I*S_J, IvarN = S_II81 - S_I^2,
             JvarN = S_JJ81 - S_J^2, cc = (crossN * rsqrt(IvarN*JvarN))^2
             accumulated per-partition via ACT Square accum_out.
  smooth     s^2 (ACT accum), lag_w / lag_h via DVE STT accum
             (lag_h uses partition-offset operands - no shift DMA).

Output per core: partial sums vector. Host assembles losses in float64.
"""
import sys
import numpy as np

sys.path.insert(0, "/opt/trn_rl_repo")

import ml_dtypes
import bass_rust
import concourse.bass as bass
import concourse.tile as tile
from concourse import mybir
from concourse import bass_utils
from concourse import tile_utils

F32 = mybir.dt.float32
BF16 = mybir.dt.bfloat16
ALU = mybir.AluOpType
ACTF = mybir.ActivationFunctionType

H = 1024
W = 1024
PAD = 4
ALPHA = 0.01
STRIDE = 120

# chunk table: (out_lo, out_n, in_lo, in_n) -- shared by h- and w-chunking
CHUNKS = []
for _c in range((H + STRIDE - 1) // STRIDE):
    _olo = STRIDE * _c
    _on = min(STRIDE, H - _olo)
    _ilo = max(0, _olo - PAD)
    _ihi = min(H, _olo + _on + PAD)
    CHUNKS.append((_olo, _on, _ilo, _ihi - _ilo))
NCH = len(CHUNKS)

# accumulator columns
CC0 = 0            # 9 cols: cc sum per w-chunk
SQ0 = 9            # 16 cols: sum s^2 per tile
LW0 = 25           # 16 cols: lag_w per tile
LH0 = 41           # 32 cols: lag_h per (tile, half)
NCOL = 73

tile_utils.max_sbuf_usage = 206 * 1024

_nc_cache = {}


def _legalize_waits(nc, max_waits=1):
    """walrus accepts only one sync-wait per instruction; split extras onto
    same-engine NoOps placed just before."""
    ctr = 0
    for f in nc.m.functions:
        for bb in f.blocks:
            insts = bb.instructions
            i = 0
            while i < len(insts):
                ins = insts[i]
                si = ins.sync_info
                if si is None:
                    i += 1
                    continue
                w = list(si.on_wait)
                if len(w) <= max_waits:
                    i += 1
                    continue
                extra, keep = w[:-max_waits], w[-max_waits:]
                nops = []
                for j in range(0, len(extra), max_waits):
                    chunk = extra[j:j + max_waits]
                    nop = mybir.InstNoOp(name=f"I-wsplit-{ctr}", ins=[], outs=[])
                    ctr += 1
                    nop.engine = ins.engine
                    nop.sync_info = bass_rust.SyncInfo(on_wait=chunk, on_update=[])
                    nops.append(nop)
                ins.sync_info = bass_rust.SyncInfo(on_wait=keep,
                                                  on_update=list(si.on_update))
                insts[i:i] = nops
                i += len(nops) + 1


def _make_host_consts():
    """Band matrices (bf16) and ones (f32). Layout [128, 4*STRIDE]:
    v0 = chunk-0 band, v1 = interior band, v0s/v1s = 81-scaled."""
    def band(klo, kn, olo, on, scale):
        k = np.arange(klo, klo + kn)[:, None]
        m = np.arange(olo, olo + on)[None, :]
        return (np.abs(k - m) <= PAD).astype(np.float32) * scale

    bands = np.zeros((128, 4 * STRIDE), dtype=np.float32)
    olo0, on0, ilo0, in0 = CHUNKS[0]
    olo1, on1, ilo1, in1 = CHUNKS[1]
    bands[:in0, 0:on0] = band(ilo0, in0, olo0, on0, 1.0)
    bands[:in1, STRIDE:STRIDE + on1] = band(ilo1, in1, olo1, on1, 1.0)
    bands[:in0, 2 * STRIDE:2 * STRIDE + on0] = band(ilo0, in0, olo0, on0, 81.0)
    bands[:in1, 3 * STRIDE:3 * STRIDE + on1] = band(ilo1, in1, olo1, on1, 81.0)
    bands_bf = bands.astype(ml_dtypes.bfloat16)
    shift = np.zeros((128, 127), dtype=np.float32)
    shift[np.arange(1, 128), np.arange(127)] = 1.0
    shift_bf = shift.astype(ml_dtypes.bfloat16)
    ones_f32 = np.ones((128, 1), dtype=np.float32)
    return bands_bf, shift_bf, ones_f32


def _band_ap(bands_t, c, scaled, inn, on):
    v = (0 if c == 0 else 1) + (2 if scaled else 0)
    return bands_t[0:inn, v * STRIDE:v * STRIDE + on]


def _build(nc):
    I_d = nc.dram_tensor("I", [H, W], F32, kind="ExternalInput").ap()
    J_d = nc.dram_tensor("J", [H, W], F32, kind="ExternalInput").ap()
    s0_d = nc.dram_tensor("s0", [H, W], F32, kind="ExternalInput").ap()
    s1_d = nc.dram_tensor("s1", [H, W], F32, kind="ExternalInput").ap()
    bands_d = nc.dram_tensor("bands", [128, 4 * STRIDE], BF16,
                             kind="ExternalInput").ap()
    shift_d = nc.dram_tensor("shift", [128, 127], BF16,
                             kind="ExternalInput").ap()
    ones_d = nc.dram_tensor("ones", [128, 1], F32, kind="ExternalInput").ap()
    part_d = nc.dram_tensor("partials", [1, NCOL], F32,
                            kind="ExternalOutput").ap()

    from contextlib import ExitStack
    with tile.TileContext(nc) as tc, ExitStack() as ctx:
        consts = ctx.enter_context(tc.tile_pool(name="consts", bufs=1))
        prod = ctx.enter_context(tc.tile_pool(name="prod", bufs=1))
        tmap = ctx.enter_context(tc.tile_pool(name="tmap", bufs=2))
        ctmp = ctx.enter_context(tc.tile_pool(name="ctmp", bufs=2))
        spool = ctx.enter_context(tc.tile_pool(name="spool", bufs=1))
        accp = ctx.enter_context(tc.tile_pool(name="accp", bufs=1))
        psA = ctx.enter_context(tc.tile_pool(name="psA", bufs=2, space="PSUM"))
        ps2 = ctx.enter_context(tc.tile_pool(name="ps2", bufs=2, space="PSUM"))

        bands_t = consts.tile([128, 4 * STRIDE], BF16)
        shift_t = consts.tile([128, 127], BF16)
        ones_t = consts.tile([128, 1], F32)
        nc.sync.dma_start(bands_t[:], bands_d)
        nc.sync.dma_start(shift_t[:], shift_d)
        nc.sync.dma_start(ones_t[:], ones_d)

        acc = accp.tile([128, NCOL], F32)
        nc.vector.memset(acc[:], 0.0)

        # ---------------- phase P: load + products (bf16) ----------------
        # maps: 0=I, 1=J (unscaled band), 2=IJ, 3=II, 4=JJ (81-scaled band)
        pmaps = [[None] * NCH for _ in range(5)]
        for ch, (olo, on, ilo, inn) in enumerate(CHUNKS):
            Ib = prod.tile([128, W], BF16, tag=f"Ib{ch}", name=f"Ib{ch}")
            Jb = prod.tile([128, W], BF16, tag=f"Jb{ch}", name=f"Jb{ch}")
            nc.gpsimd.dma_start(Ib[0:inn, :], I_d[ilo:ilo + inn, :])
            nc.gpsimd.dma_start(Jb[0:inn, :], J_d[ilo:ilo + inn, :])
            IJ = prod.tile([128, W], BF16, tag=f"IJ{ch}", name=f"IJ{ch}")
            I2 = prod.tile([128, W], BF16, tag=f"I2{ch}", name=f"I2{ch}")
            J2 = prod.tile([128, W], BF16, tag=f"J2{ch}", name=f"J2{ch}")
            nc.vector.tensor_tensor(out=IJ[0:inn, :], in0=Ib[0:inn, :],
                                    in1=Jb[0:inn, :], op=ALU.mult)
            nc.vector.tensor_tensor(out=I2[0:inn, :], in0=Ib[0:inn, :],
                                    in1=Ib[0:inn, :], op=ALU.mult)
            nc.scalar.square(J2[0:inn, :], Jb[0:inn, :])
            pmaps[0][ch] = Ib
            pmaps[1][ch] = Jb
            pmaps[2][ch] = IJ
            pmaps[3][ch] = I2
            pmaps[4][ch] = J2

        # s loads early so DMA overlaps compute; tiles persist.
        s_tiles = []
        for ch_i, s_d in enumerate((s0_d, s1_d)):
            for t in range(8):
                st = spool.tile([128, W], BF16, tag=f"s{ch_i}_{t}",
                                name=f"s{ch_i}_{t}")
                nc.gpsimd.dma_start(st[:], s_d[128 * t:128 * (t + 1), :])
                s_tiles.append(st)

        # smoothness for one s tile; lag_h via transpose-mode PE row-shift
        # (bf16 psum), STT accumulate. Interleaved into the c2 sweep.
        def emit_smooth(idx):
            st = s_tiles[idx]
            s2o = ctmp.tile([128, W], BF16, tag="junk", bufs=4, name="s2o")
            nc.scalar.activation(s2o[:], st[:], ACTF.Square,
                                 accum_out=acc[:, SQ0 + idx:SQ0 + idx + 1])
            lw = ctmp.tile([128, W], BF16, tag="junk", bufs=4, name="lw")
            nc.vector.scalar_tensor_tensor(
                out=lw[:, 0:W - 1], in0=st[:, 1:W], scalar=1.0,
                in1=st[:, 0:W - 1], op0=ALU.mult, op1=ALU.mult,
                accum_out=acc[:, LW0 + idx:LW0 + idx + 1])
            psh = ps2.tile([128, W], BF16, tag="p2f", name="psh")
            for blk in range(8):
                sl = slice(128 * blk, 128 * blk + 128)
                nc.tensor.matmul(psh[0:127, sl], shift_t[:], st[:, sl],
                                 is_transpose=True,
                                 start=(blk == 0), stop=(blk == 7),
                                 skip_group_check=True)
            lh = ctmp.tile([128, W], BF16, tag="junk", bufs=4, name="lh")
            nc.vector.scalar_tensor_tensor(
                out=lh[0:127, :], in0=psh[0:127, :], scalar=1.0,
                in1=st[0:127, :], op0=ALU.mult, op1=ALU.mult,
                accum_out=acc[0:127, LH0 + idx:LH0 + idx + 1])

        # ------------- phase AB per w-chunk c2 -------------------------
        for c2, (olo2, on2, ilo2, in2) in enumerate(CHUNKS):
            # stage A: fused H-conv + transpose -> T maps [w, h] bf16
            # one 2-bank f32 psum tile; per-bank start/stop groups
            t_tiles = []
            for m in range(5):
                pA = psA.tile([128, W], F32, tag="psA", name="pA")
                scaled = m >= 2
                first = {0: True, 1: True}
                # writes: (bank, col_lo, col_hi, ch, band_lo, band_hi)
                writes = []
                for ch, (holo, hon, hilo, hinn) in enumerate(CHUNKS):
                    lo, hi = holo, holo + hon
                    if hi <= 512 or lo >= 512:
                        writes.append((0 if hi <= 512 else 1, lo, hi, ch, 0, hon))
                    else:
                        writes.append((0, lo, 512, ch, 0, 512 - lo))
                        writes.append((1, 512, hi, ch, 512 - lo, hon))
                lastbank = {}
                for i, wr in enumerate(writes):
                    lastbank[wr[0]] = i
                for i, (bk, lo, hi, ch, blo, bhi) in enumerate(writes):
                    _, hon, hilo, hinn = CHUNKS[ch]
                    nc.tensor.matmul(
                        pA[0:in2, lo:hi],
                        pmaps[m][ch][0:hinn, ilo2:ilo2 + in2],
                        _band_ap(bands_t, ch, scaled, hinn, CHUNKS[ch][1])[:, blo:bhi],
                        start=first[bk], stop=(lastbank[bk] == i),
                        skip_group_check=True)
                    first[bk] = False
                tt = tmap.tile([128, W], BF16, tag=f"T{m}", name=f"T{m}")
                if (c2 * 5 + m) % 3 == 0:
                    nc.vector.tensor_copy(tt[0:in2, :], pA[0:in2, :])
                else:
                    nc.scalar.copy(tt[0:in2, :], pA[0:in2, :])
                t_tiles.append(tt)

            # stage B: W-conv, band stationary, T moving -> psum f32
            # combine consumes each map psum; si copied (used twice)
            bw = _band_ap(bands_t, c2, False, in2, on2)
            p2 = []
            for m in range(5):
                ph = ps2.tile([128, W], F32, tag="p2f", name="ph")
                nc.tensor.matmul(ph[0:on2, 0:512], bw, t_tiles[m][0:in2, 0:512],
                                 start=True, stop=True, skip_group_check=True)
                nc.tensor.matmul(ph[0:on2, 512:1024], bw,
                                 t_tiles[m][0:in2, 512:1024],
                                 start=True, stop=True, skip_group_check=True)
                p2.append(ph)

            n = on2
            # si copied to sbuf (read 2x); A = si^2 on ACT
            si = ctmp.tile([128, W], BF16, tag="si")
            nc.vector.tensor_copy(si[0:n, :], p2[0][0:n, :])
            A = ctmp.tile([128, W], BF16, tag="A")
            nc.scalar.square(A[0:n, :], si[0:n, :])
            # B = sj^2 straight from psum on ACT
            B = ctmp.tile([128, W], BF16, tag="B")
            nc.scalar.square(B[0:n, :], p2[1][0:n, :])
            # P = si * sj (one psum operand)
            P = ctmp.tile([128, W], BF16, tag="P")
            nc.vector.tensor_tensor(out=P[0:n, :], in0=si[0:n, :],
                                    in1=p2[1][0:n, :], op=ALU.mult)
            # crossN / IvarN / JvarN: psum - sbuf subtracts on DVE
            crossN = ctmp.tile([128, W], BF16, tag="crossN")
            nc.vector.tensor_tensor(out=crossN[0:n, :], in0=p2[2][0:n, :],
                                    in1=P[0:n, :], op=ALU.subtract)
            IvarN = ctmp.tile([128, W], BF16, tag="IvarN")
            nc.vector.tensor_tensor(out=IvarN[0:n, :], in0=p2[3][0:n, :],
                                    in1=A[0:n, :], op=ALU.subtract)
            JvarN = ctmp.tile([128, W], BF16, tag="JvarN")
            nc.vector.tensor_tensor(out=JvarN[0:n, :], in0=p2[4][0:n, :],
                                    in1=B[0:n, :], op=ALU.subtract)
            # D = IvarN * JvarN (bf16); ln kept in f32 for exp accuracy
            D = ctmp.tile([128, W], BF16, tag="D")
            nc.vector.tensor_tensor(out=D[0:n, :], in0=IvarN[0:n, :],
                                    in1=JvarN[0:n, :], op=ALU.mult)
            lnD = ctmp.tile([128, W], F32, tag="lnD")
            nc.scalar.activation(lnD[0:n, :], D[0:n, :], ACTF.Ln)
            t_r = ctmp.tile([128, W], BF16, tag="t_r")
            nc.scalar.activation(t_r[0:n, :], lnD[0:n, :], ACTF.Exp, scale=-0.5)
            u = ctmp.tile([128, W], BF16, tag="u")
            nc.vector.tensor_tensor(out=u[0:n, :], in0=crossN[0:n, :],
                                    in1=t_r[0:n, :], op=ALU.mult)
            ujunk = ctmp.tile([128, W], BF16, tag="junk", bufs=4)
            nc.scalar.activation(ujunk[0:n, :], u[0:n, :], ACTF.Square,
                                 accum_out=acc[0:n, CC0 + c2:CC0 + c2 + 1])

            if c2 < 8:
                emit_smooth(2 * c2)
                emit_smooth(2 * c2 + 1)

        # ---------------- final partition reduction ---------------------
        pF = ps2.tile([1, NCOL], F32, tag="p2f", name="pF")
        nc.tensor.matmul(pF[:], ones_t[:], acc[:], start=True, stop=True)
        outt = accp.tile([1, NCOL], F32, tag="outt")
        nc.scalar.copy(outt[:], pF[:])
        nc.sync.dma_start(part_d, outt[:])

    return


def _get_nc():
    if "nc" not in _nc_cache:
        nc = bass.Bass("TRN2", target_bir_lowering=False, debug=False)
        _build(nc)
        _legalize_waits(nc)
        _nc_cache["nc"] = nc
    return _nc_cache["nc"]


def _in_maps(I, J, s):
    B = I.shape[0]
    bands_bf, shift_bf, ones_f32 = _make_host_consts()
    in_maps = []
    for b in range(B):
        in_maps.append({
            "I": np.ascontiguousarray(I[b, 0]),
            "J": np.ascontiguousarray(J[b, 0]),
            "s0": np.ascontiguousarray(s[b, 0]),
            "s1": np.ascontiguousarray(s[b, 1]),
            "bands": bands_bf,
            "shift": shift_bf,
            "ones": ones_f32,
        })
    return in_maps


def kernel(I, J, s, sum_filt):
    B = I.shape[0]
    assert I.shape == (B, 1, H, W) and s.shape == (B, 2, H, W)
    nc = _get_nc()
    res = bass_utils.run_bass_kernel_spmd(nc, _in_maps(I, J, s),
                                          core_ids=list(range(B)))
    parts = np.stack([res.results[b]["partials"][0] for b in range(B)])
    parts = parts.astype(np.float64)

    s64 = s.astype(np.float64)
    cc_sum = float(parts[:, CC0:CC0 + 9].sum())
    s2 = parts[:, SQ0:SQ0 + 16].sum(axis=1)
    lag_w = parts[:, LW0:LW0 + 16].sum(axis=1)
    lag_h = parts[:, LH0:LH0 + 16].sum(axis=1)

    # tile-boundary lag_h pairs (rows 127/128, ...) per core
    rb = np.arange(127, H - 1, 128)
    lag_h = lag_h + (s64[:, :, rb, :] * s64[:, :, rb + 1, :]).sum(axis=(1, 2, 3))

    # edge corrections per core (both channels folded together)
    e_w = (s64[:, :, :, 0] ** 2).sum(axis=(1, 2)) + \
          (s64[:, :, :, -1] ** 2).sum(axis=(1, 2))
    e_h = (s64[:, :, 0, :] ** 2).sum(axis=(1, 2)) + \
          (s64[:, :, -1, :] ** 2).sum(axis=(1, 2))

    sum_dx2 = (2.0 * s2 - e_w - 2.0 * lag_w).sum()
    sum_dy2 = (2.0 * s2 - e_h - 2.0 * lag_h).sum()
    cnt = B * 2 * H * (W - 1)

    ncc_loss = -cc_sum / (B * H * W)
    smooth = 0.5 * (sum_dx2 / cnt + sum_dy2 / cnt) * ALPHA
    total = ncc_loss + smooth
    return np.array([total, ncc_loss, smooth], dtype=np.float32)
